# revision 22
# baseline (speedup 1.0000x reference)
"""Trainium2 Bass kernel for nn_BoundaryPredictor2 (sparse_attention).

kernel(**inputs) takes the FULL unsharded reference inputs
(hidden [8,4096,1024], pos_emb [4096,1024], Wq [1024,1024], Wk [1024,1024],
sig_temp [1], sig_thr [1], noise_u [8,4096]) and returns (pooled [S,B,D], loss),
matching the reference. Pure data parallel: one batch row per NeuronCore.

Device pipeline per core (batch row b):
  0. L[d,d'] = sum_i Wk[i,d] Wq[i,d']  (so adj[s] = h[s+1]^T . (L^T h[s]))
  1. per 512-position supertile: h = normalize(hidden+pos); PE-transpose h;
     CT[d',s] = sum_d L[d,d'] hT[d,s] (fp32r matmuls); score[s] =
     scale * sum_d' hT[d',s] CT[d',s-1] (DVE product + ones-matmul colsum).
  2. hard[s] = score[s] > tau[s]; tau is host-precomputed from noise_u by
     bisection over the reference's exact fp32 sigmoid/log chain, so the
     device only compares.
  3. segment mean pooling per 128-block: one-hot(segmb) matmul with h gives
     local segment sums/counts; partial last segments chain across blocks via
     carry matmuls; rows are divided by counts and DMA'd to a runtime row
     offset; the tail past the last segment is zero-filled.
  4. loss: device returns k_b = total boundaries; host replicates the fp32
     gammaln arithmetic.
"""

import math

import numpy as np

import concourse.bacc as bacc
import concourse.bass as bass
import concourse.mybir as mybir
from concourse.bass import ds
from concourse.bass_utils import run_bass_kernel_spmd
from concourse.expressions import smin
from concourse.tile import TileContext

B, S, D = 8, 4096, 1024
NUM_HEADS = 8
SCALE = float((D // NUM_HEADS) ** -0.5)
PRIOR = 0.2
NBLK = S // 128            # 32 position blocks
NST = S // 512             # 8 supertiles (4 blocks each)
KC = D // 128              # 8 contraction chunks
FP32 = mybir.dt.float32
FP32R = mybir.dt.float32r
INT32 = mybir.dt.int32
OP = mybir.AluOpType
ACTF = mybir.ActivationFunctionType
ENG = mybir.EngineType
N_TAIL = 20                # zero-tail tiles (covers k >= S - 20*128 - 128)
PAD = 128                  # pooled overhang pad for clamped tail writes


def r(ap):
    return ap.bitcast(FP32R)


def f(ap):
    """Read an fp32r tile as plain fp32 (for DVE/ACT use)."""
    return ap.bitcast(FP32)


def make_program():
    nc = bacc.Bacc("TRN2", target_bir_lowering=False)
    build_program(nc)
    nc.compile()
    return nc


def configure(s):
    """Shrink the sequence length for simulator testing."""
    global S, NBLK, NST
    S = s
    NBLK = S // 128
    NST = S // 512


def host_constants():
    k = np.arange(128)[:, None]
    m = np.arange(128)[None, :]
    return {
        "c_ident": np.eye(128, dtype=np.float32),
        # segmb[p] = boundaries in (0, p] : lhsT[k,m] = 1 if 1 <= k <= m
        "c_u128": ((k >= 1) & (k <= m)).astype(np.float32),
        # suffix[p] = boundaries in (p, 127] : lhsT[k,m] = 1 if k > m
        "c_v128": (k > m).astype(np.float32),
        "c_m2": np.concatenate([np.ones((128, 1), np.float32),
                                np.zeros((128, 1), np.float32)], axis=1),
        "c_r2": np.ones((128, 2), dtype=np.float32),
        "c_o2": np.ones((1, 2), dtype=np.float32),
        "c_iota": np.broadcast_to(
            np.arange(128, dtype=np.float32)[None, :], (128, 128)).copy(),
        "c_onesrow": np.ones((1, 128), dtype=np.float32),
        "c_iotacol": np.arange(128, dtype=np.float32)[:, None].copy(),
        "c_tailoff": (np.arange(128, dtype=np.float32)[:, None]
                      + 128.0 * (1.0 + np.arange(N_TAIL, dtype=np.float32))[None, :]
                      ).astype(np.float32),
    }


def ref_chain_fp32(z, thr, temp):
    """fp32 replication of reference: score -> logits(p)."""
    f = np.float32
    eps = np.finfo(np.float32).eps
    zz = ((z - f(thr)) / f(temp)).astype(np.float32)
    p = (1.0 / (1.0 + np.exp(-zz, dtype=np.float32))).astype(np.float32)
    p = np.clip(p, f(0.0), f(1.0))
    p = np.clip(p, eps, f(1.0) - eps)
    return np.log(p, dtype=np.float32) - np.log1p(-p, dtype=np.float32)


def host_tau(noise_u, thr, temp):
    """tau[b,s] such that reference hard[s] == (score[s] > tau[s]) in fp32."""
    f = np.float32
    eps = np.finfo(np.float32).eps
    u = np.clip(noise_u.astype(np.float32), eps, f(1.0) - eps)
    G = np.log(u, dtype=np.float32) - np.log1p(-u, dtype=np.float32)

    lo = np.full(noise_u.shape, -64.0, dtype=np.float64)
    hi = np.full(noise_u.shape, 64.0, dtype=np.float64)
    for _ in range(80):
        mid = (lo + hi) * 0.5
        val = ref_chain_fp32(mid.astype(np.float32), thr, temp) + G
        hi = np.where(val > 0.0, mid, hi)
        lo = np.where(val > 0.0, lo, mid)
    tau = lo.astype(np.float32)   # largest fp32-resolvable score with F+G <= 0

    # position 0: probs forced to 1.0; decide on host, encode as +-1e30
    p1 = np.clip(np.clip(f(1.0), f(0.0), f(1.0)), eps, f(1.0) - eps)
    L1 = np.log(p1, dtype=np.float32) - np.log1p(-p1, dtype=np.float32)
    hard0 = (L1 + G[:, 0]) > 0.0
    tau[:, 0] = np.where(hard0, f(-1e30), f(1e30))
    return tau


def build_program(nc):
    hid = nc.dram_tensor("hid", [S, D], FP32, kind="ExternalInput").ap()
    pos = nc.dram_tensor("pos", [S, D], FP32, kind="ExternalInput").ap()
    wq = nc.dram_tensor("wq", [D, D], FP32, kind="ExternalInput").ap()
    wk = nc.dram_tensor("wk", [D, D], FP32, kind="ExternalInput").ap()
    tau = nc.dram_tensor("tau", [128, NBLK], FP32, kind="ExternalInput").ap()
    consts = {
        name: nc.dram_tensor(name, list(arr.shape), FP32, kind="ExternalInput").ap()
        for name, arr in host_constants().items()
    }
    pooled = nc.dram_tensor("pooled", [S, D], FP32, kind="ExternalOutput").ap()
    kcnt = nc.dram_tensor("kcnt", [1, 1], FP32, kind="ExternalOutput").ap()
    score_rt = nc.dram_tensor("score_rt", [1, S], FP32).ap()
    dbg_score = nc.dram_tensor("dbg_score", [1, S], FP32, kind="ExternalOutput").ap()
    dbg_hard = nc.dram_tensor("dbg_hard", [128, NBLK], FP32, kind="ExternalOutput").ap()

    with TileContext(nc) as tc:
        _build(nc, tc, hid, pos, wq, wk, tau, consts, pooled, kcnt, score_rt,
               dbg_score, dbg_hard)
    return nc


def _build(nc, tc, hid, pos, wq, wk, tau, consts, pooled, kcnt, score_rt,
           dbg_score, dbg_hard):
    dma = nc.sync.dma_start
    score_blk = score_rt.rearrange("a (t p) -> (a p) t", p=128)  # [128, NBLK] view

    with tc.tile_pool(name="const", bufs=1) as cpool:
        ident = cpool.tile([128, 128], FP32R, tag="ident")
        u128 = cpool.tile([128, 128], FP32R, tag="u128")
        v128 = cpool.tile([128, 128], FP32R, tag="v128")
        m2 = cpool.tile([128, 2], FP32R, tag="m2")
        r2 = cpool.tile([128, 2], FP32R, tag="r2")
        o2 = cpool.tile([1, 2], FP32R, tag="o2")
        iota = cpool.tile([128, 128], FP32, tag="iota")
        onesrow = cpool.tile([1, 128], FP32R, tag="onesrow")
        iotacol = cpool.tile([128, 1], FP32, tag="iotacol")
        tailoff = cpool.tile([128, N_TAIL], FP32, tag="tailoff")
        tau_sb = cpool.tile([128, NBLK], FP32, tag="tau")
        for t_, c_ in ((ident, "c_ident"), (u128, "c_u128"), (v128, "c_v128"),
                       (m2, "c_m2"), (r2, "c_r2"), (o2, "c_o2"), (iota, "c_iota"),
                       (onesrow, "c_onesrow"), (iotacol, "c_iotacol"),
                       (tailoff, "c_tailoff")):
            dma(t_, consts[c_].bitcast(t_.dtype))
        dma(tau_sb, tau)

        with tc.tile_pool(name="lpool", bufs=1) as lpool:
            # ---- Phase 0: L[d, d'] = sum_i Wk[i,d] Wq[i,d'] ----
            l_sb = lpool.tile([128, KC, D], FP32R, tag="L")
            with tc.tile_pool(name="wpool", bufs=1) as wpool, \
                 tc.tile_pool(name="ph0ps", bufs=2, space="PSUM") as ph0ps:
                wq_sb = wpool.tile([128, KC, D], FP32R, tag="wq")
                wk_sb = wpool.tile([128, KC, D], FP32R, tag="wk")
                dma(wq_sb, wq.bitcast(FP32R).rearrange("(c p) d -> p c d", p=128))
                dma(wk_sb, wk.bitcast(FP32R).rearrange("(c p) d -> p c d", p=128))
                for m in range(KC):
                    for n in range(2):
                        ps = ph0ps.tile([128, 512], FP32, tag="ph0")
                        for c in range(KC):
                            nc.tensor.matmul(
                                ps,
                                wk_sb[:, c, m * 128:(m + 1) * 128],
                                wq_sb[:, c, n * 512:(n + 1) * 512],
                                start=(c == 0), stop=(c == KC - 1))
                        nc.scalar.copy(l_sb[:, m, n * 512:(n + 1) * 512], ps)

            with tc.tile_pool(name="hpool", bufs=2) as hpool, \
                 tc.tile_pool(name="stpool", bufs=2) as stpool, \
                 tc.tile_pool(name="smpool", bufs=1) as smpool, \
                 tc.tile_pool(name="pbig", bufs=3, space="PSUM") as pbig, \
                 tc.tile_pool(name="pcarry", bufs=1, space="PSUM") as pcarry, \
                 tc.tile_pool(name="psmall", bufs=3, space="PSUM") as psmall:

                hard_sb = smpool.tile([128, NBLK], FP32R, tag="hard")
                segmb_sb = smpool.tile([128, NBLK], FP32, tag="segmb")
                suffix_sb = smpool.tile([128, NBLK], FP32, tag="suffix")
                tot_sb = smpool.tile([1, NBLK], FP32, tag="tot")
                binc_sb = smpool.tile([1, NBLK], FP32, tag="binc")
                base_f = smpool.tile([1, NBLK], FP32, tag="basef")
                base_i = smpool.tile([1, NBLK], INT32, tag="basei")
                base_r = smpool.tile([1, NBLK], FP32R, tag="baser")
                gate_sb = smpool.tile([1, NBLK], FP32R, tag="gate")
                zrow = smpool.tile([1, 4], FP32, tag="zrow")
                scoref = smpool.tile([1, S], FP32, tag="scoref")
                carry_sb = smpool.tile([1, D], FP32R, tag="carry")
                gate2 = smpool.tile([1, 2], FP32R, tag="gate2")
                span2 = smpool.tile([1, 2], FP32R, tag="span2")
                gcc2 = smpool.tile([1, 2], FP32R, tag="gcc2")
                lastm2 = smpool.tile([128, 2], FP32R, tag="lastm2")
                ccnt_sb = smpool.tile([1, 2], FP32R, tag="ccnt")
                ztile = smpool.tile([128, D], FP32, tag="ztile")
                kc_sb = smpool.tile([1, 1], FP32, tag="kcsb")
                nc.vector.memset(zrow, 0.0)
                nc.vector.memset(ztile, 0.0)
                nc.vector.tensor_copy(gate2, zrow[:, 0:2])
                nc.vector.tensor_copy(span2, zrow[:, 0:2])
                nc.vector.tensor_copy(gcc2, zrow[:, 0:2])
                nc.vector.tensor_copy(lastm2, ztile[:, 0:2])

                h_tiles = {}
                prev_ct = None
                bchk = nc.gpsimd.to_reg(S - 1)
                for st in range(NST):
                    # ---- h + hT for 4 blocks ----
                    ht = stpool.tile([128, KC, 512], FP32R, tag="ht", bufs=1)
                    ct = stpool.tile([128, KC, 513], FP32, tag="ct")
                    for j in range(4):
                        t = st * 4 + j
                        hs = hpool.tile([128, D], FP32, tag="hs")
                        pb = hpool.tile([128, D], FP32, tag="pb")
                        dma(hs, hid[t * 128:(t + 1) * 128, :])
                        dma(pb, pos[t * 128:(t + 1) * 128, :])
                        nc.vector.tensor_tensor(hs, hs, pb, OP.add)
                        sq = hpool.tile([128, D], FP32, tag="sq", bufs=1)
                        nrm = hpool.tile([128, 4], FP32, tag="nrm")
                        nc.scalar.activation(sq, hs, ACTF.Square,
                                             accum_out=nrm[:, 0:1])
                        nc.scalar.sqrt(nrm[:, 1:2], nrm[:, 0:1])
                        nc.vector.reciprocal(nrm[:, 2:3], nrm[:, 1:2])
                        hb = hpool.tile([128, D], FP32R, tag="h", bufs=8)
                        nc.scalar.activation(hb, hs, ACTF.Copy, scale=nrm[:, 2:3])
                        h_tiles[t] = hb
                        for c in range(KC):
                            tp = pbig.tile([128, 512], FP32, tag="big")
                            nc.tensor.transpose(r(tp[:, 0:128]),
                                                hb[:, c * 128:(c + 1) * 128], ident)
                            nc.vector.tensor_copy(ht[:, c, j * 128:(j + 1) * 128],
                                                  tp[:, 0:128])

                    # ---- CT = L^T hT ----
                    for dpc in range(KC):
                        if prev_ct is None:
                            nc.vector.memset(ct[:, dpc, 0:1], 0.0)
                        else:
                            nc.vector.tensor_copy(ct[:, dpc, 0:1],
                                                  prev_ct[:, dpc, 512:513])
                        ps = pbig.tile([128, 512], FP32, tag="big")
                        for c in range(KC):
                            nc.tensor.matmul(
                                ps,
                                l_sb[:, c, dpc * 128:(dpc + 1) * 128],
                                ht[:, c, :],
                                start=(c == 0), stop=(c == KC - 1))
                        nc.vector.tensor_copy(ct[:, dpc, 1:513], ps)
                    prev_ct = ct

                    # ---- score ----
                    sps = psmall.tile([2, 512], FP32, tag="sm")
                    for dpc in range(KC):
                        pr = hpool.tile([128, 512], FP32R, tag="prod")
                        nc.vector.tensor_tensor(pr, f(ht[:, dpc, :]),
                                                ct[:, dpc, 0:512], OP.mult)
                        nc.tensor.matmul(sps, m2, pr,
                                         start=(dpc == 0), stop=(dpc == KC - 1))
                    nc.scalar.activation(scoref[:, st * 512:(st + 1) * 512],
                                         sps[0:1, :], ACTF.Copy, scale=SCALE)

                    # ---- hard/segmb/suffix/base for blocks of this supertile ----
                    cols = slice(st * 4, st * 4 + 4)
                    dma(score_rt[0:1, st * 512:(st + 1) * 512],
                        scoref[:, st * 512:(st + 1) * 512])
                    scoreb = hpool.tile([128, 4], FP32, tag="scoreb")
                    dma(scoreb, score_blk[:, cols])
                    nc.vector.tensor_tensor(hard_sb[:, cols], scoreb,
                                            tau_sb[:, cols], OP.is_gt)

                    ps_seg = psmall.tile([128, 4], FP32, tag="sm")
                    nc.tensor.matmul(ps_seg, u128, hard_sb[:, cols])
                    nc.vector.tensor_copy(segmb_sb[:, cols], ps_seg)
                    ps_suf = psmall.tile([128, 4], FP32, tag="sm")
                    nc.tensor.matmul(ps_suf, v128, hard_sb[:, cols])
                    nc.vector.tensor_copy(suffix_sb[:, cols], ps_suf)
                    ps_tot = psmall.tile([2, 4], FP32, tag="sm")
                    nc.tensor.matmul(ps_tot, m2, hard_sb[:, cols])
                    nc.scalar.copy(tot_sb[:, cols], ps_tot[0:1, :])
                    init = 0.0 if st == 0 else binc_sb[:, st * 4 - 1:st * 4]
                    nc.vector.tensor_tensor_scan(binc_sb[:, cols], tot_sb[:, cols],
                                                 zrow, init, OP.add, OP.add)
                    nc.vector.tensor_tensor(base_f[:, cols], binc_sb[:, cols],
                                            tot_sb[:, cols], OP.subtract)
                    nc.vector.tensor_tensor(base_f[:, cols], base_f[:, cols],
                                            f(hard_sb[0:1, cols]), OP.add)
                    nc.vector.tensor_scalar_add(base_f[:, cols], base_f[:, cols],
                                                -1.0)
                    nc.vector.tensor_copy(base_i[:, cols], base_f[:, cols])
                    nc.vector.tensor_copy(base_r[:, cols], base_f[:, cols])
                    nc.vector.tensor_scalar(gate_sb[:, cols], f(hard_sb[0:1, cols]),
                                            -1.0, 1.0, OP.mult, OP.add)

                    # ---- pooling for the 4 blocks ----
                    for j in range(4):
                        t = st * 4 + j
                        hb = h_tiles.pop(t)
                        oh = hpool.tile([128, 128], FP32R, tag="oh")
                        nc.vector.tensor_scalar(oh, iota, segmb_sb[:, t:t + 1],
                                                None, OP.is_equal)
                        if t > 0:
                            nc.vector.tensor_copy(gate2[:, 0:1],
                                                  f(gate_sb[:, t:t + 1]))
                            nc.vector.tensor_tensor(gcc2[:, 0:1],
                                                    f(gate_sb[:, t:t + 1]),
                                                    f(ccnt_sb[:, 0:1]), OP.mult)
                        lastm = hpool.tile([128, 1], FP32R, tag="lastm")
                        nc.vector.tensor_scalar(lastm, suffix_sb[:, t:t + 1],
                                                0.0, None, OP.is_equal)

                        pp = [pbig.tile([128, 512], FP32, tag="big",
                                        name=f"pp{t}_{n_}")
                              for n_ in range(2)]
                        ps_cnt = psmall.tile([128, 2], FP32, tag="sm")
                        for n in range(2):
                            nc.tensor.matmul(pp[n], oh,
                                             hb[:, n * 512:(n + 1) * 512])
                            if t > 0:
                                nc.tensor.matmul(
                                    pp[n][0:2, :], gate2,
                                    carry_sb[:, n * 512:(n + 1) * 512],
                                    start=False, stop=False, skip_group_check=True)
                        nc.tensor.matmul(ps_cnt, oh, r2)
                        if t > 0:
                            nc.tensor.matmul(ps_cnt[0:2, 0:2], gcc2, o2,
                                             start=False, stop=False,
                                             skip_group_check=True)

                        if t + 1 < NBLK:
                            pc = [pcarry.tile([2, 512], FP32, tag=f"carry{n_}",
                                              name=f"pc{t}_{n_}")
                                  for n_ in range(2)]
                            pcc = psmall.tile([2, 2], FP32, tag="sm")
                            nc.vector.tensor_scalar(span2[:, 0:1],
                                                    suffix_sb[0:1, t:t + 1],
                                                    0.0, None, OP.is_equal)
                            nc.vector.tensor_tensor(span2[:, 0:1],
                                                    f(span2[:, 0:1]),
                                                    f(gate_sb[:, t:t + 1]), OP.mult)
                            nc.vector.tensor_copy(lastm2[:, 0:1], f(lastm))
                            for n in range(2):
                                nc.tensor.matmul(pc[n], lastm2,
                                                 hb[:, n * 512:(n + 1) * 512])
                                if t > 0:
                                    nc.tensor.matmul(
                                        pc[n][0:2, :], span2,
                                        carry_sb[:, n * 512:(n + 1) * 512],
                                        start=False, stop=False,
                                        skip_group_check=True)
                            nc.tensor.matmul(pcc, lastm2, r2)
                            if t > 0:
                                nc.tensor.matmul(pcc[0:2, 0:2], span2, ccnt_sb,
                                                 start=False, stop=False,
                                                 skip_group_check=True)
                            for n in range(2):
                                nc.scalar.copy(carry_sb[:, n * 512:(n + 1) * 512],
                                               pc[n][0:1, :])
                            nc.scalar.copy(ccnt_sb, pcc[0:1, 0:2])

                        cntm = hpool.tile([128, 2], FP32, tag="cntm")
                        nc.vector.tensor_scalar(cntm[:, 0:1], ps_cnt[:, 0:1],
                                                1.0, None, OP.max)
                        nc.vector.reciprocal(cntm[:, 1:2], cntm[:, 0:1])
                        div = hpool.tile([128, D], FP32, tag="div")
                        for n in range(2):
                            nc.vector.tensor_scalar(div[:, n * 512:(n + 1) * 512],
                                                    pp[n], cntm[:, 1:2], None,
                                                    OP.mult)
                        # row indices base_t + p, rows >= S dropped by bounds check
                        pbx = psmall.tile([128, 2], FP32, tag="sm",
                                          name=f"pbx{t}")
                        nc.tensor.matmul(pbx, onesrow, base_r[0:1, t:t + 1].to_broadcast((1, 2)))
                        idx = hpool.tile([128, 2], FP32, tag="idx")
                        nc.vector.tensor_tensor(idx[:, 0:1], pbx[:, 0:1], iotacol, OP.add)
                        idxi = hpool.tile([128, 1], INT32, tag="idxi")
                        nc.vector.tensor_copy(idxi, idx[:, 0:1])
                        nc.gpsimd.indirect_dma_start(
                            out=pooled, in_=div, in_offset=None,
                            out_offset=bass.IndirectOffsetOnAxis(ap=idxi, axis=0),
                            bounds_check=bchk, oob_is_err=False)

                # ---- kcnt + debug + zero tail ----
                nc.scalar.copy(kc_sb, binc_sb[:, NBLK - 1:NBLK])
                dma(kcnt, kc_sb)
                dma(dbg_score[0:1, :], scoref)
                dma(dbg_hard, f(hard_sb))
                pbx31 = psmall.tile([128, N_TAIL], FP32, tag="sm")
                nc.tensor.matmul(
                    pbx31, onesrow,
                    base_r[0:1, NBLK - 1:NBLK].to_broadcast((1, N_TAIL)))
                tidxf = smpool.tile([128, N_TAIL], FP32, tag="tidxf")
                nc.vector.tensor_tensor(tidxf, pbx31, tailoff, OP.add)
                tidxi = smpool.tile([128, N_TAIL], INT32, tag="tidxi")
                nc.vector.tensor_copy(tidxi, tidxf)
                for j in range(N_TAIL):
                    nc.gpsimd.indirect_dma_start(
                        out=pooled, in_=ztile, in_offset=None,
                        out_offset=bass.IndirectOffsetOnAxis(
                            ap=tidxi[:, j:j + 1], axis=0),
                        bounds_check=bchk, oob_is_err=False)


_CACHE = {}


def _get_program():
    if "nc" not in _CACHE:
        _CACHE["nc"] = make_program()
    return _CACHE["nc"]


def kernel(hidden, pos_emb, Wq, Wk, sig_temp, sig_thr, noise_u):
    hidden = np.ascontiguousarray(np.asarray(hidden, dtype=np.float32))
    pos_emb = np.ascontiguousarray(np.asarray(pos_emb, dtype=np.float32)[:S])
    Wq = np.ascontiguousarray(np.asarray(Wq, dtype=np.float32))
    Wk = np.ascontiguousarray(np.asarray(Wk, dtype=np.float32))
    noise_u = np.asarray(noise_u, dtype=np.float32)
    thr = float(np.asarray(sig_thr).reshape(-1)[0])
    temp = float(np.asarray(sig_temp).reshape(-1)[0])
    assert hidden.shape == (B, S, D)

    tau = host_tau(noise_u, thr, temp)
    consts = host_constants()
    nc = _get_program()

    in_maps = []
    for b in range(B):
        m = dict(consts)
        m["hid"] = hidden[b]
        m["pos"] = pos_emb
        m["wq"] = Wq
        m["wk"] = Wk
        m["tau"] = np.ascontiguousarray(tau[b].reshape(NBLK, 128).T)
        in_maps.append(m)

    res = run_bass_kernel_spmd(nc, in_maps, list(range(B)),
                               **_CACHE.get("run_kwargs", {}))
    _CACHE["last_res"] = res
    outs = res.results

    pooled = np.stack([outs[b]["pooled"] for b in range(B)], axis=1)
    ks = np.array([outs[b]["kcnt"][0, 0] for b in range(B)], dtype=np.float32)
    return pooled, host_loss(ks)


def host_loss(ks):
    f = np.float32
    n = f(S)
    lg = lambda x: np.array([math.lgamma(float(v)) for v in np.atleast_1d(x)],
                            dtype=np.float32)
    log_prob = (lg(n + f(1.0)) - lg(ks + f(1.0)) - lg(n - ks + f(1.0))
                + ks * f(np.log(f(PRIOR))) + (n - ks) * f(np.log1p(f(-PRIOR))))
    return np.float32(-np.float32(np.mean(log_prob.astype(np.float32))) / n)


# revision 24
# speedup vs baseline: 1.1751x; 1.1751x over previous
"""Trainium2 Bass kernel for nn_BoundaryPredictor2 (sparse_attention).

kernel(**inputs) takes the FULL unsharded reference inputs
(hidden [8,4096,1024], pos_emb [4096,1024], Wq [1024,1024], Wk [1024,1024],
sig_temp [1], sig_thr [1], noise_u [8,4096]) and returns (pooled [S,B,D], loss),
matching the reference. Pure data parallel: one batch row per NeuronCore.

Device pipeline per core (batch row b):
  0. L[d,d'] = sum_i Wk[i,d] Wq[i,d']  (so adj[s] = h[s+1]^T . (L^T h[s]))
  1. per 512-position supertile: h = normalize(hidden+pos); PE-transpose h;
     CT[d',s] = sum_d L[d,d'] hT[d,s] (fp32r matmuls); score[s] =
     scale * sum_d' hT[d',s] CT[d',s-1] (DVE product + ones-matmul colsum).
  2. hard[s] = score[s] > tau[s]; tau is host-precomputed from noise_u by
     bisection over the reference's exact fp32 sigmoid/log chain, so the
     device only compares.
  3. segment mean pooling per 128-block: one-hot(segmb) matmul with h gives
     local segment sums/counts; partial last segments chain across blocks via
     carry matmuls; rows are divided by counts and DMA'd to a runtime row
     offset; the tail past the last segment is zero-filled.
  4. loss: device returns k_b = total boundaries; host replicates the fp32
     gammaln arithmetic.
"""

import math

import numpy as np

import concourse.bacc as bacc
import concourse.bass as bass
import concourse.mybir as mybir
from concourse.bass import ds
from concourse.bass_utils import run_bass_kernel_spmd
from concourse.expressions import smin
from concourse.tile import TileContext

B, S, D = 8, 4096, 1024
NUM_HEADS = 8
SCALE = float((D // NUM_HEADS) ** -0.5)
PRIOR = 0.2
NBLK = S // 128            # 32 position blocks
NST = S // 512             # 8 supertiles (4 blocks each)
KC = D // 128              # 8 contraction chunks
FP32 = mybir.dt.float32
FP32R = mybir.dt.float32r
INT32 = mybir.dt.int32
OP = mybir.AluOpType
ACTF = mybir.ActivationFunctionType
ENG = mybir.EngineType
N_TAIL = 20                # zero-tail tiles (covers k >= S - 20*128 - 128)
PAD = 128                  # pooled overhang pad for clamped tail writes


def r(ap):
    return ap.bitcast(FP32R)


def f(ap):
    """Read an fp32r tile as plain fp32 (for DVE/ACT use)."""
    return ap.bitcast(FP32)


def make_program():
    nc = bacc.Bacc("TRN2", target_bir_lowering=False)
    build_program(nc)
    nc.compile()
    return nc


def configure(s):
    """Shrink the sequence length for simulator testing."""
    global S, NBLK, NST
    S = s
    NBLK = S // 128
    NST = S // 512


def host_constants():
    k = np.arange(128)[:, None]
    m = np.arange(128)[None, :]
    return {
        "c_ident": np.eye(128, dtype=np.float32),
        # segmb[p] = boundaries in (0, p] : lhsT[k,m] = 1 if 1 <= k <= m
        "c_u128": ((k >= 1) & (k <= m)).astype(np.float32),
        # suffix[p] = boundaries in (p, 127] : lhsT[k,m] = 1 if k > m
        "c_v128": (k > m).astype(np.float32),
        "c_m2": np.concatenate([np.ones((128, 1), np.float32),
                                np.zeros((128, 1), np.float32)], axis=1),
        "c_r2": np.ones((128, 2), dtype=np.float32),
        "c_o2": np.ones((1, 2), dtype=np.float32),
        "c_iota": np.broadcast_to(
            np.arange(128, dtype=np.float32)[None, :], (128, 128)).copy(),
    }


def ref_chain_fp32(z, thr, temp):
    """fp32 replication of reference: score -> logits(p)."""
    f = np.float32
    eps = np.finfo(np.float32).eps
    zz = ((z - f(thr)) / f(temp)).astype(np.float32)
    p = (1.0 / (1.0 + np.exp(-zz, dtype=np.float32))).astype(np.float32)
    p = np.clip(p, f(0.0), f(1.0))
    p = np.clip(p, eps, f(1.0) - eps)
    return np.log(p, dtype=np.float32) - np.log1p(-p, dtype=np.float32)


def host_tau(noise_u, thr, temp):
    """tau[b,s] such that reference hard[s] == (score[s] > tau[s]) in fp32."""
    f = np.float32
    eps = np.finfo(np.float32).eps
    u = np.clip(noise_u.astype(np.float32), eps, f(1.0) - eps)
    G = np.log(u, dtype=np.float32) - np.log1p(-u, dtype=np.float32)

    lo = np.full(noise_u.shape, -64.0, dtype=np.float64)
    hi = np.full(noise_u.shape, 64.0, dtype=np.float64)
    for _ in range(80):
        mid = (lo + hi) * 0.5
        val = ref_chain_fp32(mid.astype(np.float32), thr, temp) + G
        hi = np.where(val > 0.0, mid, hi)
        lo = np.where(val > 0.0, lo, mid)
    tau = lo.astype(np.float32)   # largest fp32-resolvable score with F+G <= 0

    # position 0: probs forced to 1.0; decide on host, encode as +-1e30
    p1 = np.clip(np.clip(f(1.0), f(0.0), f(1.0)), eps, f(1.0) - eps)
    L1 = np.log(p1, dtype=np.float32) - np.log1p(-p1, dtype=np.float32)
    hard0 = (L1 + G[:, 0]) > 0.0
    tau[:, 0] = np.where(hard0, f(-1e30), f(1e30))
    return tau


def build_program(nc):
    hid = nc.dram_tensor("hid", [S, D], FP32, kind="ExternalInput").ap()
    pos = nc.dram_tensor("pos", [S, D], FP32, kind="ExternalInput").ap()
    wq = nc.dram_tensor("wq", [D, D], FP32, kind="ExternalInput").ap()
    wk = nc.dram_tensor("wk", [D, D], FP32, kind="ExternalInput").ap()
    tau = nc.dram_tensor("tau", [128, NBLK], FP32, kind="ExternalInput").ap()
    consts = {
        name: nc.dram_tensor(name, list(arr.shape), FP32, kind="ExternalInput").ap()
        for name, arr in host_constants().items()
    }
    pooled = nc.dram_tensor("pooled", [S + PAD, D], FP32, kind="ExternalOutput").ap()
    kcnt = nc.dram_tensor("kcnt", [1, 1], FP32, kind="ExternalOutput").ap()
    score_rt = nc.dram_tensor("score_rt", [1, S], FP32).ap()
    dbg_score = nc.dram_tensor("dbg_score", [1, S], FP32, kind="ExternalOutput").ap()
    dbg_hard = nc.dram_tensor("dbg_hard", [128, NBLK], FP32, kind="ExternalOutput").ap()

    with TileContext(nc) as tc:
        _build(nc, tc, hid, pos, wq, wk, tau, consts, pooled, kcnt, score_rt,
               dbg_score, dbg_hard)
    return nc


def _build(nc, tc, hid, pos, wq, wk, tau, consts, pooled, kcnt, score_rt,
           dbg_score, dbg_hard):
    dma = nc.sync.dma_start
    score_blk = score_rt.rearrange("a (t p) -> (a p) t", p=128)  # [128, NBLK] view

    with tc.tile_pool(name="const", bufs=1) as cpool:
        ident = cpool.tile([128, 128], FP32R, tag="ident")
        u128 = cpool.tile([128, 128], FP32R, tag="u128")
        v128 = cpool.tile([128, 128], FP32R, tag="v128")
        m2 = cpool.tile([128, 2], FP32R, tag="m2")
        r2 = cpool.tile([128, 2], FP32R, tag="r2")
        o2 = cpool.tile([1, 2], FP32R, tag="o2")
        iota = cpool.tile([128, 128], FP32, tag="iota")
        tau_sb = cpool.tile([128, NBLK], FP32, tag="tau")
        for t_, c_ in ((ident, "c_ident"), (u128, "c_u128"), (v128, "c_v128"),
                       (m2, "c_m2"), (r2, "c_r2"), (o2, "c_o2"), (iota, "c_iota")):
            dma(t_, consts[c_].bitcast(t_.dtype))
        dma(tau_sb, tau)

        with tc.tile_pool(name="lpool", bufs=1) as lpool:
            # ---- Phase 0: L[d, d'] = sum_i Wk[i,d] Wq[i,d'] ----
            l_sb = lpool.tile([128, KC, D], FP32R, tag="L")
            with tc.tile_pool(name="wpool", bufs=1) as wpool, \
                 tc.tile_pool(name="ph0ps", bufs=2, space="PSUM") as ph0ps:
                wq_sb = wpool.tile([128, KC, D], FP32R, tag="wq")
                wk_sb = wpool.tile([128, KC, D], FP32R, tag="wk")
                dma(wq_sb, wq.bitcast(FP32R).rearrange("(c p) d -> p c d", p=128))
                dma(wk_sb, wk.bitcast(FP32R).rearrange("(c p) d -> p c d", p=128))
                for m in range(KC):
                    for n in range(2):
                        ps = ph0ps.tile([128, 512], FP32, tag="ph0")
                        for c in range(KC):
                            nc.tensor.matmul(
                                ps,
                                wk_sb[:, c, m * 128:(m + 1) * 128],
                                wq_sb[:, c, n * 512:(n + 1) * 512],
                                start=(c == 0), stop=(c == KC - 1))
                        nc.scalar.copy(l_sb[:, m, n * 512:(n + 1) * 512], ps)

            with tc.tile_pool(name="hpool", bufs=2) as hpool, \
                 tc.tile_pool(name="stpool", bufs=2) as stpool, \
                 tc.tile_pool(name="smpool", bufs=1) as smpool, \
                 tc.tile_pool(name="pbig", bufs=3, space="PSUM") as pbig, \
                 tc.tile_pool(name="pcarry", bufs=1, space="PSUM") as pcarry, \
                 tc.tile_pool(name="psmall", bufs=3, space="PSUM") as psmall:

                hard_sb = smpool.tile([128, NBLK], FP32R, tag="hard")
                segmb_sb = smpool.tile([128, NBLK], FP32, tag="segmb")
                suffix_sb = smpool.tile([128, NBLK], FP32, tag="suffix")
                tot_sb = smpool.tile([1, NBLK], FP32, tag="tot")
                binc_sb = smpool.tile([1, NBLK], FP32, tag="binc")
                base_f = smpool.tile([1, NBLK], FP32, tag="basef")
                base_i = smpool.tile([1, NBLK], INT32, tag="basei")
                gate_sb = smpool.tile([1, NBLK], FP32R, tag="gate")
                zrow = smpool.tile([1, 4], FP32, tag="zrow")
                scoref = smpool.tile([1, S], FP32, tag="scoref")
                carry_sb = smpool.tile([1, D], FP32R, tag="carry")
                gate2 = smpool.tile([1, 2], FP32R, tag="gate2")
                span2 = smpool.tile([1, 2], FP32R, tag="span2")
                gcc2 = smpool.tile([1, 2], FP32R, tag="gcc2")
                lastm2 = smpool.tile([128, 2], FP32R, tag="lastm2")
                ccnt_sb = smpool.tile([1, 2], FP32R, tag="ccnt")
                ztile = smpool.tile([128, D], FP32, tag="ztile")
                kc_sb = smpool.tile([1, 1], FP32, tag="kcsb")
                nc.vector.memset(zrow, 0.0)
                nc.vector.memset(ztile, 0.0)
                nc.vector.tensor_copy(gate2, zrow[:, 0:2])
                nc.vector.tensor_copy(span2, zrow[:, 0:2])
                nc.vector.tensor_copy(gcc2, zrow[:, 0:2])
                nc.vector.tensor_copy(lastm2, ztile[:, 0:2])

                h_tiles = {}
                prev_ct = None
                btreg = nc.sync.alloc_register("btreg")
                for st in range(NST):
                    # ---- h + hT for 4 blocks ----
                    ht = stpool.tile([128, KC, 512], FP32R, tag="ht", bufs=1)
                    ct = stpool.tile([128, KC, 513], FP32, tag="ct")
                    for j in range(4):
                        t = st * 4 + j
                        hs = hpool.tile([128, D], FP32, tag="hs")
                        pb = hpool.tile([128, D], FP32, tag="pb")
                        dma(hs, hid[t * 128:(t + 1) * 128, :])
                        dma(pb, pos[t * 128:(t + 1) * 128, :])
                        nc.vector.tensor_tensor(hs, hs, pb, OP.add)
                        sq = hpool.tile([128, D], FP32, tag="sq", bufs=1)
                        nrm = hpool.tile([128, 4], FP32, tag="nrm")
                        nc.scalar.activation(sq, hs, ACTF.Square,
                                             accum_out=nrm[:, 0:1])
                        nc.scalar.sqrt(nrm[:, 1:2], nrm[:, 0:1])
                        nc.vector.reciprocal(nrm[:, 2:3], nrm[:, 1:2])
                        hb = hpool.tile([128, D], FP32R, tag="h", bufs=8)
                        nc.scalar.activation(hb, hs, ACTF.Copy, scale=nrm[:, 2:3])
                        h_tiles[t] = hb
                        for c in range(KC):
                            tp = pbig.tile([128, 512], FP32, tag="big")
                            nc.tensor.transpose(r(tp[:, 0:128]),
                                                hb[:, c * 128:(c + 1) * 128], ident)
                            nc.vector.tensor_copy(ht[:, c, j * 128:(j + 1) * 128],
                                                  tp[:, 0:128])

                    # ---- CT = L^T hT ----
                    for dpc in range(KC):
                        if prev_ct is None:
                            nc.vector.memset(ct[:, dpc, 0:1], 0.0)
                        else:
                            nc.vector.tensor_copy(ct[:, dpc, 0:1],
                                                  prev_ct[:, dpc, 512:513])
                        ps = pbig.tile([128, 512], FP32, tag="big")
                        for c in range(KC):
                            nc.tensor.matmul(
                                ps,
                                l_sb[:, c, dpc * 128:(dpc + 1) * 128],
                                ht[:, c, :],
                                start=(c == 0), stop=(c == KC - 1))
                        nc.vector.tensor_copy(ct[:, dpc, 1:513], ps)
                    prev_ct = ct

                    # ---- score ----
                    sps = psmall.tile([2, 512], FP32, tag="sm")
                    for dpc in range(KC):
                        pr = hpool.tile([128, 512], FP32R, tag="prod")
                        nc.vector.tensor_tensor(pr, f(ht[:, dpc, :]),
                                                ct[:, dpc, 0:512], OP.mult)
                        nc.tensor.matmul(sps, m2, pr,
                                         start=(dpc == 0), stop=(dpc == KC - 1))
                    nc.scalar.activation(scoref[:, st * 512:(st + 1) * 512],
                                         sps[0:1, :], ACTF.Copy, scale=SCALE)

                    # ---- hard/segmb/suffix/base for blocks of this supertile ----
                    cols = slice(st * 4, st * 4 + 4)
                    dma(score_rt[0:1, st * 512:(st + 1) * 512],
                        scoref[:, st * 512:(st + 1) * 512])
                    scoreb = hpool.tile([128, 4], FP32, tag="scoreb")
                    dma(scoreb, score_blk[:, cols])
                    nc.vector.tensor_tensor(hard_sb[:, cols], scoreb,
                                            tau_sb[:, cols], OP.is_gt)

                    ps_seg = psmall.tile([128, 4], FP32, tag="sm")
                    nc.tensor.matmul(ps_seg, u128, hard_sb[:, cols])
                    nc.vector.tensor_copy(segmb_sb[:, cols], ps_seg)
                    ps_suf = psmall.tile([128, 4], FP32, tag="sm")
                    nc.tensor.matmul(ps_suf, v128, hard_sb[:, cols])
                    nc.vector.tensor_copy(suffix_sb[:, cols], ps_suf)
                    ps_tot = psmall.tile([2, 4], FP32, tag="sm")
                    nc.tensor.matmul(ps_tot, m2, hard_sb[:, cols])
                    nc.scalar.copy(tot_sb[:, cols], ps_tot[0:1, :])
                    init = 0.0 if st == 0 else binc_sb[:, st * 4 - 1:st * 4]
                    nc.vector.tensor_tensor_scan(binc_sb[:, cols], tot_sb[:, cols],
                                                 zrow, init, OP.add, OP.add)
                    nc.vector.tensor_tensor(base_f[:, cols], binc_sb[:, cols],
                                            tot_sb[:, cols], OP.subtract)
                    nc.vector.tensor_tensor(base_f[:, cols], base_f[:, cols],
                                            f(hard_sb[0:1, cols]), OP.add)
                    nc.vector.tensor_scalar_add(base_f[:, cols], base_f[:, cols],
                                                -1.0)
                    nc.vector.tensor_copy(base_i[:, cols], base_f[:, cols])
                    nc.vector.tensor_scalar(gate_sb[:, cols], f(hard_sb[0:1, cols]),
                                            -1.0, 1.0, OP.mult, OP.add)

                    # ---- pooling for the 4 blocks ----
                    for j in range(4):
                        t = st * 4 + j
                        hb = h_tiles.pop(t)
                        oh = hpool.tile([128, 128], FP32R, tag="oh")
                        nc.vector.tensor_scalar(oh, iota, segmb_sb[:, t:t + 1],
                                                None, OP.is_equal)
                        if t > 0:
                            nc.vector.tensor_copy(gate2[:, 0:1],
                                                  f(gate_sb[:, t:t + 1]))
                            nc.vector.tensor_tensor(gcc2[:, 0:1],
                                                    f(gate_sb[:, t:t + 1]),
                                                    f(ccnt_sb[:, 0:1]), OP.mult)
                        lastm = hpool.tile([128, 1], FP32R, tag="lastm")
                        nc.vector.tensor_scalar(lastm, suffix_sb[:, t:t + 1],
                                                0.0, None, OP.is_equal)

                        pp = [pbig.tile([128, 512], FP32, tag="big",
                                        name=f"pp{t}_{n_}")
                              for n_ in range(2)]
                        ps_cnt = psmall.tile([128, 2], FP32, tag="sm")
                        for n in range(2):
                            nc.tensor.matmul(pp[n], oh,
                                             hb[:, n * 512:(n + 1) * 512])
                            if t > 0:
                                nc.tensor.matmul(
                                    pp[n][0:2, :], gate2,
                                    carry_sb[:, n * 512:(n + 1) * 512],
                                    start=False, stop=False, skip_group_check=True)
                        nc.tensor.matmul(ps_cnt, oh, r2)
                        if t > 0:
                            nc.tensor.matmul(ps_cnt[0:2, 0:2], gcc2, o2,
                                             start=False, stop=False,
                                             skip_group_check=True)

                        if t + 1 < NBLK:
                            pc = [pcarry.tile([2, 512], FP32, tag=f"carry{n_}",
                                              name=f"pc{t}_{n_}")
                                  for n_ in range(2)]
                            pcc = psmall.tile([2, 2], FP32, tag="sm")
                            nc.vector.tensor_scalar(span2[:, 0:1],
                                                    suffix_sb[0:1, t:t + 1],
                                                    0.0, None, OP.is_equal)
                            nc.vector.tensor_tensor(span2[:, 0:1],
                                                    f(span2[:, 0:1]),
                                                    f(gate_sb[:, t:t + 1]), OP.mult)
                            nc.vector.tensor_copy(lastm2[:, 0:1], f(lastm))
                            for n in range(2):
                                nc.tensor.matmul(pc[n], lastm2,
                                                 hb[:, n * 512:(n + 1) * 512])
                                if t > 0:
                                    nc.tensor.matmul(
                                        pc[n][0:2, :], span2,
                                        carry_sb[:, n * 512:(n + 1) * 512],
                                        start=False, stop=False,
                                        skip_group_check=True)
                            nc.tensor.matmul(pcc, lastm2, r2)
                            if t > 0:
                                nc.tensor.matmul(pcc[0:2, 0:2], span2, ccnt_sb,
                                                 start=False, stop=False,
                                                 skip_group_check=True)
                            for n in range(2):
                                nc.scalar.copy(carry_sb[:, n * 512:(n + 1) * 512],
                                               pc[n][0:1, :])
                            nc.scalar.copy(ccnt_sb, pcc[0:1, 0:2])

                        cntm = hpool.tile([128, 2], FP32, tag="cntm")
                        nc.vector.tensor_scalar(cntm[:, 0:1], ps_cnt[:, 0:1],
                                                1.0, None, OP.max)
                        nc.vector.reciprocal(cntm[:, 1:2], cntm[:, 0:1])
                        div = hpool.tile([128, D], FP32, tag="div")
                        for n in range(2):
                            nc.vector.tensor_scalar(div[:, n * 512:(n + 1) * 512],
                                                    pp[n], cntm[:, 1:2], None,
                                                    OP.mult)
                        nc.sync.reg_load(btreg, base_i[0:1, t:t + 1])
                        bt = nc.s_assert_within(
                            nc.snap(btreg, donate=True), 0, S - 1,
                            skip_runtime_assert=True)
                        dma(pooled[ds(bt, 128), :], div)

                # ---- kcnt + debug + zero tail ----
                nc.scalar.copy(kc_sb, binc_sb[:, NBLK - 1:NBLK])
                dma(kcnt, kc_sb)
                dma(dbg_score[0:1, :], scoref)
                dma(dbg_hard, f(hard_sb))


_CACHE = {}


def _get_program():
    if "nc" not in _CACHE:
        _CACHE["nc"] = make_program()
    return _CACHE["nc"]


def kernel(hidden, pos_emb, Wq, Wk, sig_temp, sig_thr, noise_u):
    hidden = np.ascontiguousarray(np.asarray(hidden, dtype=np.float32))
    pos_emb = np.ascontiguousarray(np.asarray(pos_emb, dtype=np.float32)[:S])
    Wq = np.ascontiguousarray(np.asarray(Wq, dtype=np.float32))
    Wk = np.ascontiguousarray(np.asarray(Wk, dtype=np.float32))
    noise_u = np.asarray(noise_u, dtype=np.float32)
    thr = float(np.asarray(sig_thr).reshape(-1)[0])
    temp = float(np.asarray(sig_temp).reshape(-1)[0])
    assert hidden.shape == (B, S, D)

    tau = host_tau(noise_u, thr, temp)
    consts = host_constants()
    nc = _get_program()

    in_maps = []
    for b in range(B):
        m = dict(consts)
        m["hid"] = hidden[b]
        m["pos"] = pos_emb
        m["wq"] = Wq
        m["wk"] = Wk
        m["tau"] = np.ascontiguousarray(tau[b].reshape(NBLK, 128).T)
        in_maps.append(m)

    res = run_bass_kernel_spmd(nc, in_maps, list(range(B)),
                               **_CACHE.get("run_kwargs", {}))
    _CACHE["last_res"] = res
    outs = res.results

    pooled = np.stack([outs[b]["pooled"][:S] for b in range(B)], axis=1)
    ks = np.array([outs[b]["kcnt"][0, 0] for b in range(B)], dtype=np.float32)
    return pooled, host_loss(ks)


def host_loss(ks):
    f = np.float32
    n = f(S)
    lg = lambda x: np.array([math.lgamma(float(v)) for v in np.atleast_1d(x)],
                            dtype=np.float32)
    log_prob = (lg(n + f(1.0)) - lg(ks + f(1.0)) - lg(n - ks + f(1.0))
                + ks * f(np.log(f(PRIOR))) + (n - ks) * f(np.log1p(f(-PRIOR))))
    return np.float32(-np.float32(np.mean(log_prob.astype(np.float32))) / n)


# revision 28
# speedup vs baseline: 119.9607x; 102.0893x over previous
"""Trainium2 Bass kernel for nn_BoundaryPredictor2 (sparse_attention).

kernel(**inputs) takes the FULL unsharded reference inputs
(hidden [8,4096,1024], pos_emb [4096,1024], Wq [1024,1024], Wk [1024,1024],
sig_temp [1], sig_thr [1], noise_u [8,4096]) and returns (pooled [S,B,D], loss),
matching the reference. Pure data parallel: one batch row per NeuronCore.

Device pipeline per core (batch row b):
  0. L[d,d'] = sum_i Wk[i,d] Wq[i,d']  (so adj[s] = h[s+1]^T . (L^T h[s]))
  1. per 512-position supertile: h = normalize(hidden+pos); PE-transpose h;
     CT[d',s] = sum_d L[d,d'] hT[d,s] (fp32r matmuls); score[s] =
     scale * sum_d' hT[d',s] CT[d',s-1] (DVE product + ones-matmul colsum).
  2. hard[s] = score[s] > tau[s]; tau is host-precomputed from noise_u by
     bisection over the reference's exact fp32 sigmoid/log chain, so the
     device only compares.
  3. segment mean pooling per 128-block: one-hot(segmb) matmul with h gives
     local segment sums/counts; partial last segments chain across blocks via
     carry matmuls; rows are divided by counts and DMA'd to a runtime row
     offset; the tail past the last segment is zero-filled.
  4. loss: device returns k_b = total boundaries; host replicates the fp32
     gammaln arithmetic.
"""

import math

import numpy as np

import concourse.bacc as bacc
import concourse.bass as bass
import concourse.mybir as mybir
from concourse.bass import ds
from concourse.bass_utils import run_bass_kernel_spmd
from concourse.tile import TileContext

B, S, D = 8, 4096, 1024
NUM_HEADS = 8
SCALE = float((D // NUM_HEADS) ** -0.5)
PRIOR = 0.2
NBLK = S // 128            # 32 position blocks
NST = S // 512             # 8 supertiles (4 blocks each)
KC = D // 128              # 8 contraction chunks
FP32 = mybir.dt.float32
FP32R = mybir.dt.float32r
INT32 = mybir.dt.int32
OP = mybir.AluOpType
ACTF = mybir.ActivationFunctionType
ENG = mybir.EngineType
PAD = 128   # pooled rows pad: last block may overhang past S; host slices [:S]


def r(ap):
    return ap.bitcast(FP32R)


def f(ap):
    """Read an fp32r tile as plain fp32 (for DVE/ACT use)."""
    return ap.bitcast(FP32)


def make_program(repeat=1):
    nc = bacc.Bacc("TRN2", target_bir_lowering=False)
    build_program(nc, repeat=repeat)
    nc.compile()
    return nc


def configure(s):
    """Shrink the sequence length for simulator testing."""
    global S, NBLK, NST
    S = s
    NBLK = S // 128
    NST = S // 512


def host_constants():
    k = np.arange(128)[:, None]
    m = np.arange(128)[None, :]
    return {
        "c_ident": np.eye(128, dtype=np.float32),
        # segmb[p] = boundaries in (0, p] : lhsT[k,m] = 1 if 1 <= k <= m
        "c_u128": ((k >= 1) & (k <= m)).astype(np.float32),
        # suffix[p] = boundaries in (p, 127] : lhsT[k,m] = 1 if k > m
        "c_v128": (k > m).astype(np.float32),
        "c_m2": np.concatenate([np.ones((128, 1), np.float32),
                                np.zeros((128, 1), np.float32)], axis=1),
        "c_r2": np.ones((128, 2), dtype=np.float32),
        "c_o2": np.ones((1, 2), dtype=np.float32),
        "c_iota": np.broadcast_to(
            np.arange(128, dtype=np.float32)[None, :], (128, 128)).copy(),
    }


def ref_chain_fp32(z, thr, temp):
    """fp32 replication of reference: score -> logits(p)."""
    f = np.float32
    eps = np.finfo(np.float32).eps
    zz = ((z - f(thr)) / f(temp)).astype(np.float32)
    p = (1.0 / (1.0 + np.exp(-zz, dtype=np.float32))).astype(np.float32)
    p = np.clip(p, f(0.0), f(1.0))
    p = np.clip(p, eps, f(1.0) - eps)
    return np.log(p, dtype=np.float32) - np.log1p(-p, dtype=np.float32)


def host_tau(noise_u, thr, temp):
    """tau[b,s] such that reference hard[s] == (score[s] > tau[s]) in fp32."""
    f = np.float32
    eps = np.finfo(np.float32).eps
    u = np.clip(noise_u.astype(np.float32), eps, f(1.0) - eps)
    G = np.log(u, dtype=np.float32) - np.log1p(-u, dtype=np.float32)

    lo = np.full(noise_u.shape, -64.0, dtype=np.float64)
    hi = np.full(noise_u.shape, 64.0, dtype=np.float64)
    for _ in range(80):
        mid = (lo + hi) * 0.5
        val = ref_chain_fp32(mid.astype(np.float32), thr, temp) + G
        hi = np.where(val > 0.0, mid, hi)
        lo = np.where(val > 0.0, lo, mid)
    tau = lo.astype(np.float32)   # largest fp32-resolvable score with F+G <= 0

    # position 0: probs forced to 1.0; decide on host, encode as +-1e30
    p1 = np.clip(np.clip(f(1.0), f(0.0), f(1.0)), eps, f(1.0) - eps)
    L1 = np.log(p1, dtype=np.float32) - np.log1p(-p1, dtype=np.float32)
    hard0 = (L1 + G[:, 0]) > 0.0
    tau[:, 0] = np.where(hard0, f(-1e30), f(1e30))
    return tau


def build_program(nc, repeat=1):
    hid = nc.dram_tensor("hid", [S, D], FP32, kind="ExternalInput").ap()
    pos = nc.dram_tensor("pos", [S, D], FP32, kind="ExternalInput").ap()
    wq = nc.dram_tensor("wq", [D, D], FP32, kind="ExternalInput").ap()
    wk = nc.dram_tensor("wk", [D, D], FP32, kind="ExternalInput").ap()
    tau = nc.dram_tensor("tau", [128, NBLK], FP32, kind="ExternalInput").ap()
    consts = {
        name: nc.dram_tensor(name, list(arr.shape), FP32, kind="ExternalInput").ap()
        for name, arr in host_constants().items()
    }
    pooled = nc.dram_tensor("pooled", [S + PAD, D], FP32, kind="ExternalOutput").ap()
    kcnt = nc.dram_tensor("kcnt", [1, 1], FP32, kind="ExternalOutput").ap()
    score_rt = nc.dram_tensor("score_rt", [1, S], FP32).ap()
    dbg_score = nc.dram_tensor("dbg_score", [1, S], FP32, kind="ExternalOutput").ap()
    dbg_hard = nc.dram_tensor("dbg_hard", [128, NBLK], FP32, kind="ExternalOutput").ap()

    with TileContext(nc) as tc:
        for _ in range(repeat):
            _build(nc, tc, hid, pos, wq, wk, tau, consts, pooled, kcnt, score_rt,
                   dbg_score, dbg_hard)
    return nc


def _build(nc, tc, hid, pos, wq, wk, tau, consts, pooled, kcnt, score_rt,
           dbg_score, dbg_hard):
    dma = nc.sync.dma_start
    score_blk = score_rt.rearrange("a (t p) -> (a p) t", p=128)  # [128, NBLK] view

    with tc.tile_pool(name="const", bufs=1) as cpool:
        ident = cpool.tile([128, 128], FP32R, tag="ident")
        u128 = cpool.tile([128, 128], FP32R, tag="u128")
        v128 = cpool.tile([128, 128], FP32R, tag="v128")
        m2 = cpool.tile([128, 2], FP32R, tag="m2")
        r2 = cpool.tile([128, 2], FP32R, tag="r2")
        o2 = cpool.tile([1, 2], FP32R, tag="o2")
        iota = cpool.tile([128, 128], FP32, tag="iota")
        tau_sb = cpool.tile([128, NBLK], FP32, tag="tau")
        for t_, c_ in ((ident, "c_ident"), (u128, "c_u128"), (v128, "c_v128"),
                       (m2, "c_m2"), (r2, "c_r2"), (o2, "c_o2"), (iota, "c_iota")):
            dma(t_, consts[c_].bitcast(t_.dtype))
        dma(tau_sb, tau)

        with tc.tile_pool(name="lpool", bufs=1) as lpool:
            # ---- Phase 0: L[d, d'] = sum_i Wk[i,d] Wq[i,d'] ----
            l_sb = lpool.tile([128, KC, D], FP32R, tag="L")
            with tc.tile_pool(name="wpool", bufs=1) as wpool, \
                 tc.tile_pool(name="ph0ps", bufs=2, space="PSUM") as ph0ps:
                wq_sb = wpool.tile([128, KC, D], FP32R, tag="wq")
                wk_sb = wpool.tile([128, KC, D], FP32R, tag="wk")
                wqr = wq.bitcast(FP32R).rearrange("(c p) d -> p c d", p=128)
                wkr = wk.bitcast(FP32R).rearrange("(c p) d -> p c d", p=128)
                for c in range(KC):
                    dma(wq_sb[:, c, :], wqr[:, c, :])
                    dma(wk_sb[:, c, :], wkr[:, c, :])
                for m in range(KC):
                    for n in range(2):
                        ps = ph0ps.tile([128, 512], FP32, tag="ph0")
                        for c in range(KC):
                            nc.tensor.matmul(
                                ps,
                                wk_sb[:, c, m * 128:(m + 1) * 128],
                                wq_sb[:, c, n * 512:(n + 1) * 512],
                                start=(c == 0), stop=(c == KC - 1))
                        nc.scalar.copy(l_sb[:, m, n * 512:(n + 1) * 512], ps)

            with tc.tile_pool(name="hpool", bufs=2) as hpool, \
                 tc.tile_pool(name="stpool", bufs=2) as stpool, \
                 tc.tile_pool(name="smpool", bufs=1) as smpool, \
                 tc.tile_pool(name="pbig", bufs=2, space="PSUM") as pbig, \
                 tc.tile_pool(name="pcarry", bufs=1, space="PSUM") as pcarry, \
                 tc.tile_pool(name="psmall", bufs=2, space="PSUM") as psmall:

                hard_sb = smpool.tile([128, NBLK], FP32R, tag="hard")
                segmb_sb = smpool.tile([128, NBLK], FP32, tag="segmb")
                suffix_sb = smpool.tile([128, NBLK], FP32, tag="suffix")
                tot_sb = smpool.tile([1, NBLK], FP32, tag="tot")
                binc_sb = smpool.tile([1, NBLK], FP32, tag="binc")
                base_f = smpool.tile([1, NBLK], FP32, tag="basef")
                base_i = smpool.tile([1, NBLK], INT32, tag="basei")
                gate_sb = smpool.tile([1, NBLK], FP32R, tag="gate")
                zrow = smpool.tile([1, 4], FP32, tag="zrow")
                scoref = smpool.tile([1, S], FP32, tag="scoref")
                carry_sb = smpool.tile([1, D], FP32R, tag="carry")
                gate2 = smpool.tile([1, 2], FP32R, tag="gate2")
                span2 = smpool.tile([1, 2], FP32R, tag="span2")
                gcc2 = smpool.tile([1, 2], FP32R, tag="gcc2")
                lastm2 = smpool.tile([128, 2], FP32R, tag="lastm2")
                ccnt_sb = smpool.tile([1, 2], FP32R, tag="ccnt")
                ztile = smpool.tile([128, D], FP32, tag="ztile")
                kc_sb = smpool.tile([1, 1], FP32, tag="kcsb")
                nc.vector.memset(zrow, 0.0)
                nc.vector.memset(ztile, 0.0)
                nc.vector.tensor_copy(gate2, zrow[:, 0:2])
                nc.vector.tensor_copy(span2, zrow[:, 0:2])
                nc.vector.tensor_copy(gcc2, zrow[:, 0:2])
                nc.vector.tensor_copy(lastm2, ztile[:, 0:2])

                h_tiles = {}
                prev_ct = None
                btreg = nc.sync.alloc_register(f"btreg{nc.next_id()}")
                for st in range(NST):
                    # ---- h + hT for 4 blocks ----
                    ht = stpool.tile([128, KC, 512], FP32R, tag="ht", bufs=1)
                    ct = stpool.tile([128, KC, 513], FP32, tag="ct")
                    for j in range(4):
                        t = st * 4 + j
                        hs = hpool.tile([128, D], FP32, tag="hs")
                        pb = hpool.tile([128, D], FP32, tag="pb")
                        dma(hs, hid[t * 128:(t + 1) * 128, :])
                        dma(pb, pos[t * 128:(t + 1) * 128, :])
                        nc.vector.tensor_tensor(hs, hs, pb, OP.add)
                        sq = hpool.tile([128, D], FP32, tag="sq", bufs=1)
                        nrm = hpool.tile([128, 4], FP32, tag="nrm")
                        nc.scalar.activation(sq, hs, ACTF.Square,
                                             accum_out=nrm[:, 0:1])
                        nc.scalar.sqrt(nrm[:, 1:2], nrm[:, 0:1])
                        nc.vector.reciprocal(nrm[:, 2:3], nrm[:, 1:2])
                        hb = hpool.tile([128, D], FP32R, tag="h", bufs=8)
                        nc.scalar.activation(hb, hs, ACTF.Copy, scale=nrm[:, 2:3])
                        h_tiles[t] = hb
                        for c in range(KC):
                            tp = pbig.tile([128, 512], FP32, tag="big")
                            nc.tensor.transpose(r(tp[:, 0:128]),
                                                hb[:, c * 128:(c + 1) * 128], ident)
                            nc.scalar.copy(ht[:, c, j * 128:(j + 1) * 128],
                                           tp[:, 0:128])

                    # ---- CT = L^T hT ----
                    for dpc in range(KC):
                        if prev_ct is None:
                            nc.vector.memset(ct[:, dpc, 0:1], 0.0)
                        else:
                            nc.vector.tensor_copy(ct[:, dpc, 0:1],
                                                  prev_ct[:, dpc, 512:513])
                        ps = pbig.tile([128, 512], FP32, tag="big")
                        for c in range(KC):
                            nc.tensor.matmul(
                                ps,
                                l_sb[:, c, dpc * 128:(dpc + 1) * 128],
                                ht[:, c, :],
                                start=(c == 0), stop=(c == KC - 1))
                        nc.vector.tensor_copy(ct[:, dpc, 1:513], ps)
                    prev_ct = ct

                    # ---- score ----
                    sps = psmall.tile([2, 512], FP32, tag="sm")
                    for dpc in range(KC):
                        pr = hpool.tile([128, 512], FP32R, tag="prod")
                        nc.vector.tensor_tensor(pr, f(ht[:, dpc, :]),
                                                ct[:, dpc, 0:512], OP.mult)
                        nc.tensor.matmul(sps, m2, pr,
                                         start=(dpc == 0), stop=(dpc == KC - 1))
                    nc.scalar.activation(scoref[:, st * 512:(st + 1) * 512],
                                         sps[0:1, :], ACTF.Copy, scale=SCALE)

                    # ---- hard/segmb/suffix/base for blocks of this supertile ----
                    cols = slice(st * 4, st * 4 + 4)
                    dma(score_rt[0:1, st * 512:(st + 1) * 512],
                        scoref[:, st * 512:(st + 1) * 512])
                    scoreb = hpool.tile([128, 4], FP32, tag="scoreb")
                    dma(scoreb, score_blk[:, cols])
                    nc.vector.tensor_tensor(hard_sb[:, cols], scoreb,
                                            tau_sb[:, cols], OP.is_gt)

                    ps_seg = psmall.tile([128, 4], FP32, tag="sm")
                    nc.tensor.matmul(ps_seg, u128, hard_sb[:, cols])
                    nc.vector.tensor_copy(segmb_sb[:, cols], ps_seg)
                    ps_suf = psmall.tile([128, 4], FP32, tag="sm")
                    nc.tensor.matmul(ps_suf, v128, hard_sb[:, cols])
                    nc.vector.tensor_copy(suffix_sb[:, cols], ps_suf)
                    ps_tot = psmall.tile([2, 4], FP32, tag="sm")
                    nc.tensor.matmul(ps_tot, m2, hard_sb[:, cols])
                    nc.scalar.copy(tot_sb[:, cols], ps_tot[0:1, :])
                    init = 0.0 if st == 0 else binc_sb[:, st * 4 - 1:st * 4]
                    nc.vector.tensor_tensor_scan(binc_sb[:, cols], tot_sb[:, cols],
                                                 zrow, init, OP.add, OP.add)
                    nc.vector.tensor_tensor(base_f[:, cols], binc_sb[:, cols],
                                            tot_sb[:, cols], OP.subtract)
                    nc.vector.tensor_tensor(base_f[:, cols], base_f[:, cols],
                                            f(hard_sb[0:1, cols]), OP.add)
                    nc.vector.tensor_scalar_add(base_f[:, cols], base_f[:, cols],
                                                -1.0)
                    nc.vector.tensor_copy(base_i[:, cols], base_f[:, cols])
                    nc.vector.tensor_scalar(gate_sb[:, cols], f(hard_sb[0:1, cols]),
                                            -1.0, 1.0, OP.mult, OP.add)

                    # ---- pooling for the 4 blocks ----
                    for j in range(4):
                        t = st * 4 + j
                        hb = h_tiles.pop(t)
                        oh = hpool.tile([128, 128], FP32R, tag="oh")
                        nc.vector.tensor_scalar(oh, iota, segmb_sb[:, t:t + 1],
                                                None, OP.is_equal)
                        if t > 0:
                            nc.vector.tensor_copy(gate2[:, 0:1],
                                                  f(gate_sb[:, t:t + 1]))
                            nc.vector.tensor_tensor(gcc2[:, 0:1],
                                                    f(gate_sb[:, t:t + 1]),
                                                    f(ccnt_sb[:, 0:1]), OP.mult)
                        lastm = hpool.tile([128, 1], FP32R, tag="lastm")
                        nc.vector.tensor_scalar(lastm, suffix_sb[:, t:t + 1],
                                                0.0, None, OP.is_equal)

                        pp = [pbig.tile([128, 512], FP32, tag="pool", bufs=2,
                                        name=f"pp{t}_{n_}")
                              for n_ in range(2)]
                        ps_cnt = psmall.tile([128, 2], FP32, tag="sm")
                        for n in range(2):
                            nc.tensor.matmul(pp[n], oh,
                                             hb[:, n * 512:(n + 1) * 512])
                            if t > 0:
                                nc.tensor.matmul(
                                    pp[n][0:2, :], gate2,
                                    carry_sb[:, n * 512:(n + 1) * 512],
                                    start=False, stop=False, skip_group_check=True)
                        nc.tensor.matmul(ps_cnt, oh, r2)
                        if t > 0:
                            nc.tensor.matmul(ps_cnt[0:2, 0:2], gcc2, o2,
                                             start=False, stop=False,
                                             skip_group_check=True)

                        if t + 1 < NBLK:
                            pc = [pcarry.tile([2, 512], FP32, tag=f"carry{n_}",
                                              name=f"pc{t}_{n_}")
                                  for n_ in range(2)]
                            pcc = psmall.tile([2, 2], FP32, tag="sm")
                            nc.vector.tensor_scalar(span2[:, 0:1],
                                                    suffix_sb[0:1, t:t + 1],
                                                    0.0, None, OP.is_equal)
                            nc.vector.tensor_tensor(span2[:, 0:1],
                                                    f(span2[:, 0:1]),
                                                    f(gate_sb[:, t:t + 1]), OP.mult)
                            nc.vector.tensor_copy(lastm2[:, 0:1], f(lastm))
                            for n in range(2):
                                nc.tensor.matmul(pc[n], lastm2,
                                                 hb[:, n * 512:(n + 1) * 512])
                                if t > 0:
                                    nc.tensor.matmul(
                                        pc[n][0:2, :], span2,
                                        carry_sb[:, n * 512:(n + 1) * 512],
                                        start=False, stop=False,
                                        skip_group_check=True)
                            nc.tensor.matmul(pcc, lastm2, r2)
                            if t > 0:
                                nc.tensor.matmul(pcc[0:2, 0:2], span2, ccnt_sb,
                                                 start=False, stop=False,
                                                 skip_group_check=True)
                            for n in range(2):
                                nc.scalar.copy(carry_sb[:, n * 512:(n + 1) * 512],
                                               pc[n][0:1, :])
                            nc.scalar.copy(ccnt_sb, pcc[0:1, 0:2])

                        cntm = hpool.tile([128, 2], FP32, tag="cntm")
                        nc.vector.tensor_scalar(cntm[:, 0:1], ps_cnt[:, 0:1],
                                                1.0, None, OP.max)
                        nc.vector.reciprocal(cntm[:, 1:2], cntm[:, 0:1])
                        div = hpool.tile([128, D], FP32, tag="div")
                        for n in range(2):
                            nc.vector.tensor_scalar(div[:, n * 512:(n + 1) * 512],
                                                    pp[n], cntm[:, 1:2], None,
                                                    OP.mult)
                        nc.sync.reg_load(btreg, base_i[0:1, t:t + 1])
                        bt = nc.s_assert_within(
                            nc.snap(btreg, donate=True), 0, S - 1,
                            skip_runtime_assert=True)
                        dma(pooled[ds(bt, 128), :], div)

                # ---- kcnt + debug + zero tail ----
                nc.scalar.copy(kc_sb, binc_sb[:, NBLK - 1:NBLK])
                dma(kcnt, kc_sb)
                dma(dbg_score[0:1, :], scoref)
                dma(dbg_hard, f(hard_sb))


_CACHE = {}


def _get_program():
    if "nc" not in _CACHE:
        _CACHE["nc"] = make_program()
    return _CACHE["nc"]


def kernel(hidden, pos_emb, Wq, Wk, sig_temp, sig_thr, noise_u):
    hidden = np.ascontiguousarray(np.asarray(hidden, dtype=np.float32))
    pos_emb = np.ascontiguousarray(np.asarray(pos_emb, dtype=np.float32)[:S])
    Wq = np.ascontiguousarray(np.asarray(Wq, dtype=np.float32))
    Wk = np.ascontiguousarray(np.asarray(Wk, dtype=np.float32))
    noise_u = np.asarray(noise_u, dtype=np.float32)
    thr = float(np.asarray(sig_thr).reshape(-1)[0])
    temp = float(np.asarray(sig_temp).reshape(-1)[0])
    assert hidden.shape == (B, S, D)

    tau = host_tau(noise_u, thr, temp)
    consts = host_constants()
    nc = _get_program()

    in_maps = []
    for b in range(B):
        m = dict(consts)
        m["hid"] = hidden[b]
        m["pos"] = pos_emb
        m["wq"] = Wq
        m["wk"] = Wk
        m["tau"] = np.ascontiguousarray(tau[b].reshape(NBLK, 128).T)
        in_maps.append(m)

    res = run_bass_kernel_spmd(nc, in_maps, list(range(B)),
                               **_CACHE.get("run_kwargs", {}))
    _CACHE["last_res"] = res
    outs = res.results

    pooled = np.stack([outs[b]["pooled"][:S] for b in range(B)], axis=1)
    ks = np.array([outs[b]["kcnt"][0, 0] for b in range(B)], dtype=np.float32)
    return pooled, host_loss(ks)


def host_loss(ks):
    f = np.float32
    n = f(S)
    lg = lambda x: np.array([math.lgamma(float(v)) for v in np.atleast_1d(x)],
                            dtype=np.float32)
    log_prob = (lg(n + f(1.0)) - lg(ks + f(1.0)) - lg(n - ks + f(1.0))
                + ks * f(np.log(f(PRIOR))) + (n - ks) * f(np.log1p(f(-PRIOR))))
    return np.float32(-np.float32(np.mean(log_prob.astype(np.float32))) / n)


# revision 29
# speedup vs baseline: 126.8459x; 1.0574x over previous
"""Trainium2 Bass kernel for nn_BoundaryPredictor2 (sparse_attention).

kernel(**inputs) takes the FULL unsharded reference inputs
(hidden [8,4096,1024], pos_emb [4096,1024], Wq [1024,1024], Wk [1024,1024],
sig_temp [1], sig_thr [1], noise_u [8,4096]) and returns (pooled [S,B,D], loss),
matching the reference. Pure data parallel: one batch row per NeuronCore.

Device pipeline per core (batch row b):
  0. L[d,d'] = sum_i Wk[i,d] Wq[i,d']  (so adj[s] = h[s+1]^T . (L^T h[s]))
  1. per 512-position supertile: h = normalize(hidden+pos); PE-transpose h;
     CT[d',s] = sum_d L[d,d'] hT[d,s] (fp32r matmuls); score[s] =
     scale * sum_d' hT[d',s] CT[d',s-1] (DVE product + ones-matmul colsum).
  2. hard[s] = score[s] > tau[s]; tau is host-precomputed from noise_u by
     bisection over the reference's exact fp32 sigmoid/log chain, so the
     device only compares.
  3. segment mean pooling per 128-block: one-hot(segmb) matmul with h gives
     local segment sums/counts; partial last segments chain across blocks via
     carry matmuls; rows are divided by counts and DMA'd to a runtime row
     offset; the tail past the last segment is zero-filled.
  4. loss: device returns k_b = total boundaries; host replicates the fp32
     gammaln arithmetic.
"""

import math

import numpy as np

import concourse.bacc as bacc
import concourse.bass as bass
import concourse.mybir as mybir
from concourse.bass import ds
from concourse.bass_utils import run_bass_kernel_spmd
from concourse.tile import TileContext

B, S, D = 8, 4096, 1024
NUM_HEADS = 8
SCALE = float((D // NUM_HEADS) ** -0.5)
PRIOR = 0.2
NBLK = S // 128            # 32 position blocks
NST = S // 512             # 8 supertiles (4 blocks each)
KC = D // 128              # 8 contraction chunks
FP32 = mybir.dt.float32
FP32R = mybir.dt.float32r
INT32 = mybir.dt.int32
OP = mybir.AluOpType
ACTF = mybir.ActivationFunctionType
ENG = mybir.EngineType
PAD = 128   # pooled rows pad: last block may overhang past S; host slices [:S]


def r(ap):
    return ap.bitcast(FP32R)


def f(ap):
    """Read an fp32r tile as plain fp32 (for DVE/ACT use)."""
    return ap.bitcast(FP32)


def make_program(repeat=1):
    nc = bacc.Bacc("TRN2", target_bir_lowering=False)
    build_program(nc, repeat=repeat)
    nc.compile()
    return nc


def configure(s):
    """Shrink the sequence length for simulator testing."""
    global S, NBLK, NST
    S = s
    NBLK = S // 128
    NST = S // 512


def host_constants():
    k = np.arange(128)[:, None]
    m = np.arange(128)[None, :]
    return {
        "c_ident": np.eye(128, dtype=np.float32),
        # segmb[p] = boundaries in (0, p] : lhsT[k,m] = 1 if 1 <= k <= m
        "c_u128": ((k >= 1) & (k <= m)).astype(np.float32),
        # suffix[p] = boundaries in (p, 127] : lhsT[k,m] = 1 if k > m
        "c_v128": (k > m).astype(np.float32),
        "c_m2": np.concatenate([np.ones((128, 1), np.float32),
                                np.zeros((128, 1), np.float32)], axis=1),
        "c_r2": np.ones((128, 2), dtype=np.float32),
        "c_o2": np.ones((1, 2), dtype=np.float32),
        "c_iota": np.broadcast_to(
            np.arange(128, dtype=np.float32)[None, :], (128, 128)).copy(),
    }


def ref_chain_fp32(z, thr, temp):
    """fp32 replication of reference: score -> logits(p)."""
    f = np.float32
    eps = np.finfo(np.float32).eps
    zz = ((z - f(thr)) / f(temp)).astype(np.float32)
    p = (1.0 / (1.0 + np.exp(-zz, dtype=np.float32))).astype(np.float32)
    p = np.clip(p, f(0.0), f(1.0))
    p = np.clip(p, eps, f(1.0) - eps)
    return np.log(p, dtype=np.float32) - np.log1p(-p, dtype=np.float32)


def host_tau(noise_u, thr, temp):
    """tau[b,s] such that reference hard[s] == (score[s] > tau[s]) in fp32."""
    f = np.float32
    eps = np.finfo(np.float32).eps
    u = np.clip(noise_u.astype(np.float32), eps, f(1.0) - eps)
    G = np.log(u, dtype=np.float32) - np.log1p(-u, dtype=np.float32)

    lo = np.full(noise_u.shape, -64.0, dtype=np.float64)
    hi = np.full(noise_u.shape, 64.0, dtype=np.float64)
    for _ in range(80):
        mid = (lo + hi) * 0.5
        val = ref_chain_fp32(mid.astype(np.float32), thr, temp) + G
        hi = np.where(val > 0.0, mid, hi)
        lo = np.where(val > 0.0, lo, mid)
    tau = lo.astype(np.float32)   # largest fp32-resolvable score with F+G <= 0

    # position 0: probs forced to 1.0; decide on host, encode as +-1e30
    p1 = np.clip(np.clip(f(1.0), f(0.0), f(1.0)), eps, f(1.0) - eps)
    L1 = np.log(p1, dtype=np.float32) - np.log1p(-p1, dtype=np.float32)
    hard0 = (L1 + G[:, 0]) > 0.0
    tau[:, 0] = np.where(hard0, f(-1e30), f(1e30))
    return tau


def build_program(nc, repeat=1):
    hid = nc.dram_tensor("hid", [S, D], FP32, kind="ExternalInput").ap()
    pos = nc.dram_tensor("pos", [S, D], FP32, kind="ExternalInput").ap()
    wq = nc.dram_tensor("wq", [D, D], FP32, kind="ExternalInput").ap()
    wk = nc.dram_tensor("wk", [D, D], FP32, kind="ExternalInput").ap()
    tau = nc.dram_tensor("tau", [128, NBLK], FP32, kind="ExternalInput").ap()
    consts = {
        name: nc.dram_tensor(name, list(arr.shape), FP32, kind="ExternalInput").ap()
        for name, arr in host_constants().items()
    }
    pooled = nc.dram_tensor("pooled", [S + PAD, D], FP32, kind="ExternalOutput").ap()
    kcnt = nc.dram_tensor("kcnt", [1, 1], FP32, kind="ExternalOutput").ap()
    score_rt = nc.dram_tensor("score_rt", [1, S], FP32).ap()
    dbg_score = nc.dram_tensor("dbg_score", [1, S], FP32, kind="ExternalOutput").ap()
    dbg_hard = nc.dram_tensor("dbg_hard", [128, NBLK], FP32, kind="ExternalOutput").ap()

    with TileContext(nc) as tc:
        for _ in range(repeat):
            _build(nc, tc, hid, pos, wq, wk, tau, consts, pooled, kcnt, score_rt,
                   dbg_score, dbg_hard)
    return nc


def _build(nc, tc, hid, pos, wq, wk, tau, consts, pooled, kcnt, score_rt,
           dbg_score, dbg_hard):
    dma = nc.sync.dma_start
    score_blk = score_rt.rearrange("a (t p) -> (a p) t", p=128)  # [128, NBLK] view

    with tc.tile_pool(name="const", bufs=1) as cpool:
        ident = cpool.tile([128, 128], FP32R, tag="ident")
        u128 = cpool.tile([128, 128], FP32R, tag="u128")
        v128 = cpool.tile([128, 128], FP32R, tag="v128")
        m2 = cpool.tile([128, 2], FP32R, tag="m2")
        r2 = cpool.tile([128, 2], FP32R, tag="r2")
        o2 = cpool.tile([1, 2], FP32R, tag="o2")
        iota = cpool.tile([128, 128], FP32, tag="iota")
        tau_sb = cpool.tile([128, NBLK], FP32, tag="tau")
        for t_, c_ in ((ident, "c_ident"), (u128, "c_u128"), (v128, "c_v128"),
                       (m2, "c_m2"), (r2, "c_r2"), (o2, "c_o2"), (iota, "c_iota")):
            dma(t_, consts[c_].bitcast(t_.dtype))
        dma(tau_sb, tau)

        with tc.tile_pool(name="lpool", bufs=1) as lpool:
            # ---- Phase 0: L[d, d'] = sum_i Wk[i,d] Wq[i,d'] ----
            l_sb = lpool.tile([128, KC, D], FP32R, tag="L")
            with tc.tile_pool(name="wpool", bufs=1) as wpool, \
                 tc.tile_pool(name="ph0ps", bufs=2, space="PSUM") as ph0ps:
                wq_sb = wpool.tile([128, KC, D], FP32R, tag="wq")
                wk_sb = wpool.tile([128, KC, D], FP32R, tag="wk")
                wqr = wq.bitcast(FP32R).rearrange("(c p) d -> p c d", p=128)
                wkr = wk.bitcast(FP32R).rearrange("(c p) d -> p c d", p=128)
                for c in range(KC):
                    dma(wq_sb[:, c, :], wqr[:, c, :])
                    dma(wk_sb[:, c, :], wkr[:, c, :])
                for m in range(KC):
                    for n in range(2):
                        ps = ph0ps.tile([128, 512], FP32, tag="ph0")
                        for c in range(KC):
                            nc.tensor.matmul(
                                ps,
                                wk_sb[:, c, m * 128:(m + 1) * 128],
                                wq_sb[:, c, n * 512:(n + 1) * 512],
                                start=(c == 0), stop=(c == KC - 1))
                        nc.scalar.copy(l_sb[:, m, n * 512:(n + 1) * 512], ps)

            with tc.tile_pool(name="hpool", bufs=2) as hpool, \
                 tc.tile_pool(name="stpool", bufs=2) as stpool, \
                 tc.tile_pool(name="smpool", bufs=1) as smpool, \
                 tc.tile_pool(name="pbig", bufs=2, space="PSUM") as pbig, \
                 tc.tile_pool(name="pcarry", bufs=1, space="PSUM") as pcarry, \
                 tc.tile_pool(name="psmall", bufs=2, space="PSUM") as psmall:

                hard_sb = smpool.tile([128, NBLK], FP32R, tag="hard")
                segmb_sb = smpool.tile([128, NBLK], FP32, tag="segmb")
                suffix_sb = smpool.tile([128, NBLK], FP32, tag="suffix")
                tot_sb = smpool.tile([1, NBLK], FP32, tag="tot")
                binc_sb = smpool.tile([1, NBLK], FP32, tag="binc")
                base_f = smpool.tile([1, NBLK], FP32, tag="basef")
                base_i = smpool.tile([1, NBLK], INT32, tag="basei")
                gate_sb = smpool.tile([1, NBLK], FP32R, tag="gate")
                zrow = smpool.tile([1, 4], FP32, tag="zrow")
                scoref = smpool.tile([1, S], FP32, tag="scoref")
                carry_sb = smpool.tile([1, D], FP32R, tag="carry")
                gate2 = smpool.tile([1, 2], FP32R, tag="gate2")
                span2 = smpool.tile([1, 2], FP32R, tag="span2")
                gcc2 = smpool.tile([1, 2], FP32R, tag="gcc2")
                lastm2 = smpool.tile([128, 2], FP32R, tag="lastm2")
                ccnt_sb = smpool.tile([1, 2], FP32R, tag="ccnt")
                ztile = smpool.tile([128, D], FP32, tag="ztile")
                kc_sb = smpool.tile([1, 1], FP32, tag="kcsb")
                nc.vector.memset(zrow, 0.0)
                nc.vector.memset(ztile, 0.0)
                nc.vector.tensor_copy(gate2, zrow[:, 0:2])
                nc.vector.tensor_copy(span2, zrow[:, 0:2])
                nc.vector.tensor_copy(gcc2, zrow[:, 0:2])
                nc.vector.tensor_copy(lastm2, ztile[:, 0:2])

                h_tiles = {}
                prev_ct = None
                btreg = nc.sync.alloc_register(f"btreg{nc.next_id()}")
                for st in range(NST):
                    # ---- h + hT for 4 blocks ----
                    ht = stpool.tile([128, KC, 512], FP32R, tag="ht", bufs=2)
                    ct = stpool.tile([128, KC, 513], FP32, tag="ct")
                    for j in range(4):
                        t = st * 4 + j
                        hs = hpool.tile([128, D], FP32, tag="hs", bufs=3)
                        pb = hpool.tile([128, D], FP32, tag="pb", bufs=3)
                        dma(hs, hid[t * 128:(t + 1) * 128, :])
                        dma(pb, pos[t * 128:(t + 1) * 128, :])
                        nc.vector.tensor_tensor(hs, hs, pb, OP.add)
                        sq = hpool.tile([128, D], FP32, tag="sq", bufs=1)
                        nrm = hpool.tile([128, 4], FP32, tag="nrm")
                        nc.scalar.activation(sq, hs, ACTF.Square,
                                             accum_out=nrm[:, 0:1])
                        nc.scalar.sqrt(nrm[:, 1:2], nrm[:, 0:1])
                        nc.vector.reciprocal(nrm[:, 2:3], nrm[:, 1:2])
                        hb = hpool.tile([128, D], FP32R, tag="h", bufs=8)
                        nc.scalar.activation(hb, hs, ACTF.Copy, scale=nrm[:, 2:3])
                        h_tiles[t] = hb
                        for c in range(KC):
                            tp = pbig.tile([128, 512], FP32, tag="big")
                            nc.tensor.transpose(r(tp[:, 0:128]),
                                                hb[:, c * 128:(c + 1) * 128], ident)
                            nc.scalar.copy(ht[:, c, j * 128:(j + 1) * 128],
                                           tp[:, 0:128])

                    # ---- CT = L^T hT ----
                    for dpc in range(KC):
                        if prev_ct is None:
                            nc.vector.memset(ct[:, dpc, 0:1], 0.0)
                        else:
                            nc.vector.tensor_copy(ct[:, dpc, 0:1],
                                                  prev_ct[:, dpc, 512:513])
                        ps = pbig.tile([128, 512], FP32, tag="big")
                        for c in range(KC):
                            nc.tensor.matmul(
                                ps,
                                l_sb[:, c, dpc * 128:(dpc + 1) * 128],
                                ht[:, c, :],
                                start=(c == 0), stop=(c == KC - 1))
                        nc.vector.tensor_copy(ct[:, dpc, 1:513], ps)
                    prev_ct = ct

                    # ---- score ----
                    sps = psmall.tile([2, 512], FP32, tag="sm")
                    for dpc in range(KC):
                        pr = hpool.tile([128, 512], FP32R, tag="prod")
                        nc.vector.tensor_tensor(pr, f(ht[:, dpc, :]),
                                                ct[:, dpc, 0:512], OP.mult)
                        nc.tensor.matmul(sps, m2, pr,
                                         start=(dpc == 0), stop=(dpc == KC - 1))
                    nc.scalar.activation(scoref[:, st * 512:(st + 1) * 512],
                                         sps[0:1, :], ACTF.Copy, scale=SCALE)

                    # ---- hard/segmb/suffix/base for blocks of this supertile ----
                    cols = slice(st * 4, st * 4 + 4)
                    dma(score_rt[0:1, st * 512:(st + 1) * 512],
                        scoref[:, st * 512:(st + 1) * 512])
                    scoreb = hpool.tile([128, 4], FP32, tag="scoreb")
                    dma(scoreb, score_blk[:, cols])
                    nc.vector.tensor_tensor(hard_sb[:, cols], scoreb,
                                            tau_sb[:, cols], OP.is_gt)

                    ps_seg = psmall.tile([128, 4], FP32, tag="sm")
                    nc.tensor.matmul(ps_seg, u128, hard_sb[:, cols])
                    nc.vector.tensor_copy(segmb_sb[:, cols], ps_seg)
                    ps_suf = psmall.tile([128, 4], FP32, tag="sm")
                    nc.tensor.matmul(ps_suf, v128, hard_sb[:, cols])
                    nc.vector.tensor_copy(suffix_sb[:, cols], ps_suf)
                    ps_tot = psmall.tile([2, 4], FP32, tag="sm")
                    nc.tensor.matmul(ps_tot, m2, hard_sb[:, cols])
                    nc.scalar.copy(tot_sb[:, cols], ps_tot[0:1, :])
                    init = 0.0 if st == 0 else binc_sb[:, st * 4 - 1:st * 4]
                    nc.vector.tensor_tensor_scan(binc_sb[:, cols], tot_sb[:, cols],
                                                 zrow, init, OP.add, OP.add)
                    nc.vector.tensor_tensor(base_f[:, cols], binc_sb[:, cols],
                                            tot_sb[:, cols], OP.subtract)
                    nc.vector.tensor_tensor(base_f[:, cols], base_f[:, cols],
                                            f(hard_sb[0:1, cols]), OP.add)
                    nc.vector.tensor_scalar_add(base_f[:, cols], base_f[:, cols],
                                                -1.0)
                    nc.vector.tensor_copy(base_i[:, cols], base_f[:, cols])
                    nc.vector.tensor_scalar(gate_sb[:, cols], f(hard_sb[0:1, cols]),
                                            -1.0, 1.0, OP.mult, OP.add)

                    # ---- pooling for the 4 blocks ----
                    for j in range(4):
                        t = st * 4 + j
                        hb = h_tiles.pop(t)
                        oh = hpool.tile([128, 128], FP32R, tag="oh")
                        nc.vector.tensor_scalar(oh, iota, segmb_sb[:, t:t + 1],
                                                None, OP.is_equal)
                        if t > 0:
                            nc.vector.tensor_copy(gate2[:, 0:1],
                                                  f(gate_sb[:, t:t + 1]))
                            nc.vector.tensor_tensor(gcc2[:, 0:1],
                                                    f(gate_sb[:, t:t + 1]),
                                                    f(ccnt_sb[:, 0:1]), OP.mult)
                        lastm = hpool.tile([128, 1], FP32R, tag="lastm")
                        nc.vector.tensor_scalar(lastm, suffix_sb[:, t:t + 1],
                                                0.0, None, OP.is_equal)

                        pp = [pbig.tile([128, 512], FP32, tag="pool", bufs=2,
                                        name=f"pp{t}_{n_}")
                              for n_ in range(2)]
                        ps_cnt = psmall.tile([128, 2], FP32, tag="sm")
                        for n in range(2):
                            nc.tensor.matmul(pp[n], oh,
                                             hb[:, n * 512:(n + 1) * 512])
                            if t > 0:
                                nc.tensor.matmul(
                                    pp[n][0:2, :], gate2,
                                    carry_sb[:, n * 512:(n + 1) * 512],
                                    start=False, stop=False, skip_group_check=True)
                        nc.tensor.matmul(ps_cnt, oh, r2)
                        if t > 0:
                            nc.tensor.matmul(ps_cnt[0:2, 0:2], gcc2, o2,
                                             start=False, stop=False,
                                             skip_group_check=True)

                        if t + 1 < NBLK:
                            pc = [pcarry.tile([2, 512], FP32, tag=f"carry{n_}",
                                              name=f"pc{t}_{n_}")
                                  for n_ in range(2)]
                            pcc = psmall.tile([2, 2], FP32, tag="sm")
                            nc.vector.tensor_scalar(span2[:, 0:1],
                                                    suffix_sb[0:1, t:t + 1],
                                                    0.0, None, OP.is_equal)
                            nc.vector.tensor_tensor(span2[:, 0:1],
                                                    f(span2[:, 0:1]),
                                                    f(gate_sb[:, t:t + 1]), OP.mult)
                            nc.vector.tensor_copy(lastm2[:, 0:1], f(lastm))
                            for n in range(2):
                                nc.tensor.matmul(pc[n], lastm2,
                                                 hb[:, n * 512:(n + 1) * 512])
                                if t > 0:
                                    nc.tensor.matmul(
                                        pc[n][0:2, :], span2,
                                        carry_sb[:, n * 512:(n + 1) * 512],
                                        start=False, stop=False,
                                        skip_group_check=True)
                            nc.tensor.matmul(pcc, lastm2, r2)
                            if t > 0:
                                nc.tensor.matmul(pcc[0:2, 0:2], span2, ccnt_sb,
                                                 start=False, stop=False,
                                                 skip_group_check=True)
                            for n in range(2):
                                nc.scalar.copy(carry_sb[:, n * 512:(n + 1) * 512],
                                               pc[n][0:1, :])
                            nc.scalar.copy(ccnt_sb, pcc[0:1, 0:2])

                        cntm = hpool.tile([128, 2], FP32, tag="cntm")
                        nc.vector.tensor_scalar(cntm[:, 0:1], ps_cnt[:, 0:1],
                                                1.0, None, OP.max)
                        nc.vector.reciprocal(cntm[:, 1:2], cntm[:, 0:1])
                        div = hpool.tile([128, D], FP32, tag="div")
                        for n in range(2):
                            nc.vector.tensor_scalar(div[:, n * 512:(n + 1) * 512],
                                                    pp[n], cntm[:, 1:2], None,
                                                    OP.mult)
                        nc.sync.reg_load(btreg, base_i[0:1, t:t + 1])
                        bt = nc.s_assert_within(
                            nc.snap(btreg, donate=True), 0, S - 1,
                            skip_runtime_assert=True)
                        dma(pooled[ds(bt, 128), :], div)

                # ---- kcnt + debug + zero tail ----
                nc.scalar.copy(kc_sb, binc_sb[:, NBLK - 1:NBLK])
                dma(kcnt, kc_sb)
                dma(dbg_score[0:1, :], scoref)
                dma(dbg_hard, f(hard_sb))


_CACHE = {}


def _get_program():
    if "nc" not in _CACHE:
        _CACHE["nc"] = make_program()
    return _CACHE["nc"]


def kernel(hidden, pos_emb, Wq, Wk, sig_temp, sig_thr, noise_u):
    hidden = np.ascontiguousarray(np.asarray(hidden, dtype=np.float32))
    pos_emb = np.ascontiguousarray(np.asarray(pos_emb, dtype=np.float32)[:S])
    Wq = np.ascontiguousarray(np.asarray(Wq, dtype=np.float32))
    Wk = np.ascontiguousarray(np.asarray(Wk, dtype=np.float32))
    noise_u = np.asarray(noise_u, dtype=np.float32)
    thr = float(np.asarray(sig_thr).reshape(-1)[0])
    temp = float(np.asarray(sig_temp).reshape(-1)[0])
    assert hidden.shape == (B, S, D)

    tau = host_tau(noise_u, thr, temp)
    consts = host_constants()
    nc = _get_program()

    in_maps = []
    for b in range(B):
        m = dict(consts)
        m["hid"] = hidden[b]
        m["pos"] = pos_emb
        m["wq"] = Wq
        m["wk"] = Wk
        m["tau"] = np.ascontiguousarray(tau[b].reshape(NBLK, 128).T)
        in_maps.append(m)

    res = run_bass_kernel_spmd(nc, in_maps, list(range(B)),
                               **_CACHE.get("run_kwargs", {}))
    _CACHE["last_res"] = res
    outs = res.results

    pooled = np.stack([outs[b]["pooled"][:S] for b in range(B)], axis=1)
    ks = np.array([outs[b]["kcnt"][0, 0] for b in range(B)], dtype=np.float32)
    return pooled, host_loss(ks)


def host_loss(ks):
    f = np.float32
    n = f(S)
    lg = lambda x: np.array([math.lgamma(float(v)) for v in np.atleast_1d(x)],
                            dtype=np.float32)
    log_prob = (lg(n + f(1.0)) - lg(ks + f(1.0)) - lg(n - ks + f(1.0))
                + ks * f(np.log(f(PRIOR))) + (n - ks) * f(np.log1p(f(-PRIOR))))
    return np.float32(-np.float32(np.mean(log_prob.astype(np.float32))) / n)
